# revision 28
# baseline (speedup 1.0000x reference)
"""CasRel-style kernel for Trainium2 (Bass/Tile), 8-core data-parallel.

Model (per batch b):
  hn_b      = masked LSTM over embed[b, head_b : head_b+16, :]  (16 steps, H=768)
  w_b       = hn_b @ cln_Ww.T + cln_weight ; bvec_b = hn_b @ cln_Wb.T + cln_bias
  ne[b,s]   = w_b * (x - mu)/std + bvec_b          (layernorm over H)
  heads     = sigmoid(ne @ Wh.T + bh) ; tails = sigmoid(ne @ Wt.T + bt)

Kernel strategy (per core, 8 local batches):
  - LN + classifier folded into one PSUM accumulation on raw x:
      logits = a_s * (x_s @ Wc_b.T - mu_s x vh_b + std_s x c_b),  a_s = 1/std_s
    where Wc_b = [Wh|Wt] col-scaled by w_b, vh_b = Wc_b @ 1,
    c_b = [Wh|Wt] @ bvec_b + [bh|bt]. The mu/std rank-1 corrections ride into
    PSUM as K=1 matmuls (mu/std as bf16 rows); a_s rides the sigmoid scale.
  - All matmul operands bf16; weights cast to bf16 during DMA (SWDGE) and
    transposed on-chip by the PE (bf16 transpose -> bf16 PSUM -> one batched
    copy per 768 columns). No xbar DMA transposes anywhere.
  - All HBM reads go through the gpsimd SWDGE queue in priority order
    (gather, W_ih, W_hh, Wh/Wt, x tiles, cln_Ww/Wb) so the recurrence's
    weights are never starved by embed traffic.
  - LSTM: x-part pre-GEMMed for all 16 steps; per-step h @ W_hh.T uses two
    PSUM tiles (i/f/g and o) so the i/f/g activations overlap the o matmuls.
  - x-tile pipeline (stats + PE transposes) interleaved 2 tiles per LSTM step.
"""

import functools
import os
import sys

import numpy as np

sys.path.insert(0, "/opt/trn_rl_repo")

import concourse.bass as bass
import concourse.tile as tile
from concourse import bacc, mybir
from concourse.bass import IndirectOffsetOnAxis
from concourse.bass_utils import run_bass_kernel_spmd
from concourse.masks import make_identity

F32 = mybir.dt.float32
BF16 = mybir.dt.bfloat16
I32 = mybir.dt.int32

B, S, H, R = 64, 512, 768, 51
T = 16                      # MAX_SPAN
NCORES = 8
BL = B // NCORES            # 8 local batches
KC = H // 128               # 6 contraction chunks
GM = 4 * H // 128           # 24 gate-dim chunks
NT = BL * S // 128          # 32 token tiles per core
R2 = 2 * R                  # 102 combined head+tail outputs
EPS = 1e-12


def _kernel_body(tc):
    nc = tc.nc
    embed = nc.dram_tensor("embed", [BL, S, H], F32, kind="ExternalInput").ap()
    sub_head = nc.dram_tensor("sub_head", [BL], I32, kind="ExternalInput").ap()
    sub_tail = nc.dram_tensor("sub_tail", [BL], I32, kind="ExternalInput").ap()
    W_ih = nc.dram_tensor("W_ih", [4 * H, H], F32, kind="ExternalInput").ap()
    W_hh = nc.dram_tensor("W_hh", [4 * H, H], F32, kind="ExternalInput").ap()
    b_ih = nc.dram_tensor("b_ih", [4 * H], F32, kind="ExternalInput").ap()
    b_hh = nc.dram_tensor("b_hh", [4 * H], F32, kind="ExternalInput").ap()
    cln_w = nc.dram_tensor("cln_weight", [H], F32, kind="ExternalInput").ap()
    cln_b = nc.dram_tensor("cln_bias", [H], F32, kind="ExternalInput").ap()
    cln_Ww = nc.dram_tensor("cln_Ww", [H, H], F32, kind="ExternalInput").ap()
    cln_Wb = nc.dram_tensor("cln_Wb", [H, H], F32, kind="ExternalInput").ap()
    Wh = nc.dram_tensor("Wh", [R, H], F32, kind="ExternalInput").ap()
    bh = nc.dram_tensor("bh", [R], F32, kind="ExternalInput").ap()
    Wt = nc.dram_tensor("Wt", [R, H], F32, kind="ExternalInput").ap()
    bt = nc.dram_tensor("bt", [R], F32, kind="ExternalInput").ap()
    heads = nc.dram_tensor("heads", [BL, S, R], F32, kind="ExternalOutput").ap()
    tails = nc.dram_tensor("tails", [BL, S, R], F32, kind="ExternalOutput").ap()

    Sig = mybir.ActivationFunctionType.Sigmoid
    Tanh = mybir.ActivationFunctionType.Tanh
    Sqrt = mybir.ActivationFunctionType.Sqrt
    Copy = mybir.ActivationFunctionType.Copy
    NKB = KC * BL  # 48

    # alternate PSUM->SBUF copies between the two streaming engines
    _ccnt = [0]

    def psum_copy(dst, src):
        if _ccnt[0] % 2 == 0:
            nc.scalar.activation(dst, src, Copy)
        else:
            nc.vector.tensor_copy(dst, src)
        _ccnt[0] += 1

    with (
        tc.tile_pool(name="persist", bufs=1) as pp,
        tc.tile_pool(name="wstage", bufs=3) as wstage,
        tc.tile_pool(name="xstage", bufs=2) as xstage,
        tc.tile_pool(name="ostage", bufs=2) as ostage,
    ):
        # ---------------- constants / small prep ----------------
        identb = pp.tile([128, 128], BF16, tag="identb")
        make_identity(nc, identb[:])

        ones_row = pp.tile([1, 128], BF16, tag="ones_row")
        nc.vector.memset(ones_row[:], 1.0)

        head_sb = pp.tile([BL, 1], I32, tag="head_sb")
        tail_sb = pp.tile([BL, 1], I32, tag="tail_sb")
        nc.sync.dma_start(head_sb[:], sub_head.rearrange("b -> b ()"))
        nc.sync.dma_start(tail_sb[:], sub_tail.rearrange("b -> b ()"))

        # gather row offsets: off[b, t] = b*S + head_b + t  (embed flat [BL*S, H])
        iota_bs = pp.tile([BL, T], I32, tag="iota_bs")
        nc.gpsimd.iota(iota_bs[:], pattern=[[1, T]], base=0, channel_multiplier=S)
        off_bt = pp.tile([BL, T], I32, tag="off_bt")
        nc.vector.tensor_tensor(off_bt[:], iota_bs[:],
                                head_sb[:, 0:1].to_broadcast([BL, T]),
                                op=mybir.AluOpType.add)
        off_p = pp.tile([128, 1], I32, tag="off_p")
        nc.sync.dma_start(off_p[:], off_bt[:])

        # span mask: mask[b, t] = (t <= tail_b - head_b)
        span = pp.tile([BL, 1], I32, tag="span")
        nc.vector.tensor_tensor(span[:], tail_sb[:], head_sb[:],
                                op=mybir.AluOpType.subtract)
        iota_t = pp.tile([BL, T], I32, tag="iota_t")
        nc.gpsimd.iota(iota_t[:], pattern=[[1, T]], base=0, channel_multiplier=0)
        mask_pad = pp.tile([32, 32], I32, tag="mask_pad")
        nc.vector.memset(mask_pad[:], 0)
        nc.vector.tensor_tensor(mask_pad[0:BL, 0:T], iota_t[:],
                                span[:, 0:1].to_broadcast([BL, T]),
                                op=mybir.AluOpType.is_le)
        maskTt = pp.tile([32, 32], I32, tag="maskTt")      # [t, b]
        nc.vector.transpose(maskTt[:], mask_pad[:])
        maskT = pp.tile([1, T * BL], I32, tag="maskT")     # col = t*8+b
        nc.sync.dma_start(maskT[0:1, :], maskTt[0:T, 0:BL])
        mask_bc = pp.tile([128, T * BL], I32, tag="mask_bc")
        nc.gpsimd.partition_broadcast(mask_bc[:], maskT[0:1, :])
        # replicate over h-chunks: mask_bc6[p, t*48 + k*8 + b] = mask[b, t]
        mask_bc6 = pp.tile([128, T * KC * BL], mybir.dt.uint8, tag="mask_bc6")
        m6v = mask_bc6[:, :].rearrange("p (t k b) -> p k t b", k=KC, b=BL)
        mbv = mask_bc[:, :].rearrange("p (t b) -> p t b", b=BL)
        for k in range(KC):
            nc.vector.tensor_copy(m6v[:, k], mbv)

        # chunk-major small vectors: v_c[p, m] = v[m*128 + p]
        clnw_c = pp.tile([128, KC], F32, tag="clnw_c")
        clnb_c = pp.tile([128, KC], F32, tag="clnb_c")
        nc.sync.dma_start(clnw_c[:], cln_w.rearrange("(k p) -> p k", p=128))
        nc.sync.dma_start(clnb_c[:], cln_b.rearrange("(k p) -> p k", p=128))
        brow_c = pp.tile([128, GM], F32, tag="brow_c")
        nc.gpsimd.dma_start(brow_c[:], b_ih.rearrange("(m p) -> p m", p=128))
        nc.gpsimd.dma_start(brow_c[:], b_hh.rearrange("(m p) -> p m", p=128),
                            accum_op=mybir.AluOpType.add)
        bhbt_f = pp.tile([1, R2], F32, tag="bhbt_f")
        nc.sync.dma_start(bhbt_f[:, 0:R], bh.rearrange("r -> () r"))
        nc.sync.dma_start(bhbt_f[:, R:R2], bt.rearrange("r -> () r"))
        bhbt = pp.tile([1, R2], BF16, tag="bhbt")
        nc.vector.tensor_copy(bhbt[:], bhbt_f[:])

        # span gather (f32) -> bf16 for the pre-GEMM rhs
        xsp_f = xstage.tile([128, H], F32, tag="xsp_f")
        nc.gpsimd.indirect_dma_start(
            out=xsp_f[:], out_offset=None,
            in_=embed.rearrange("b s h -> (b s) h"),
            in_offset=IndirectOffsetOnAxis(ap=off_p[:, 0:1], axis=0))
        xsp_b = xstage.tile([128, H], BF16, tag="xsp_b")
        nc.vector.tensor_copy(xsp_b[:], xsp_f[:])

        # persistent big tensors
        WihT = pp.tile([128, KC, 4 * H], BF16, tag="WihT")
        WhhT = pp.tile([128, KC, 4 * H], BF16, tag="WhhT")
        xT_all = pp.tile([128, NT, KC, 128], BF16, tag="xT_all")
        gates0 = pp.tile([128, GM * 128], BF16, tag="gates0")
        g0r = gates0[:, :].rearrange("p (m b t) -> p m b t", m=GM, b=BL)
        xspT = pp.tile([128, KC, 128], BF16, tag="xspT")
        whwt = pp.tile([128, KC, R2], BF16, tag="whwt")
        wt_tp = pp.tile([128, KC, 128], BF16, tag="wt_tp")

        # LSTM state
        hbf = pp.tile([128, NKB], BF16, tag="hbf")   # col = k*8 + b
        cT = pp.tile([128, NKB], F32, tag="cT")
        gsig = pp.tile([128, 4 * NKB], F32, tag="gsig")  # i|f|o|tanh_g
        gates_f = pp.tile([128, 4 * NKB], F32, tag="gates_f")
        tmp1 = pp.tile([128, NKB], F32, tag="tmp1")
        tmp2 = pp.tile([128, NKB], F32, tag="tmp2")
        tmph = pp.tile([128, NKB], BF16, tag="tmph")
        nc.vector.memset(hbf[:], 0.0)
        nc.vector.memset(cT[:], 0.0)

        # stats: col i = mu_i, col NT+i = var_i (later std_i)
        stats = pp.tile([128, 2 * NT], F32, tag="stats")
        a_all = pp.tile([128, NT], F32, tag="a_all")
        eps_t = pp.tile([128, 1], F32, tag="eps_t")
        nc.vector.memset(eps_t[:], EPS)

        with (
            tc.tile_pool(name="psum_tp", bufs=2, space="PSUM") as ptp,
            tc.tile_pool(name="psum_rec", bufs=2, space="PSUM") as psr,
        ):
            # -------- weight load (bf16 cast in DMA) + PE transpose --------
            def load_wT(dst, src_ap, nm, stag):
                # dst[:, k, m*128 + c] = src[m*128 + c, k*128 + p]
                for m0 in range(0, nm, 2):
                    st = wstage.tile([128, 2, H], BF16, tag=stag)
                    nc.gpsimd.dma_start(
                        st[:, :, :],
                        src_ap[m0 * 128:(m0 + 2) * 128, :]
                        .rearrange("(c p) h -> p c h", p=128))
                    for c in range(2):
                        ps = ptp.tile([128, H], BF16, tag="tp")
                        for k in range(KC):
                            nc.tensor.transpose(ps[:, k * 128:(k + 1) * 128],
                                                st[:, c, k * 128:(k + 1) * 128],
                                                identb[:])
                        m = m0 + c
                        psum_copy(
                            dst[:, :, m * 128:(m + 1) * 128],
                            ps[:].rearrange("p (k c) -> p k c", k=KC))

            # W_ih first (pre-GEMM gates t=0), then W_hh (recurrence t>=1)
            load_wT(WihT, W_ih, GM, "wstg")

            # xspT: 6 transposes of the gathered span block
            ps = ptp.tile([128, H], BF16, tag="tp")
            for k in range(KC):
                nc.tensor.transpose(ps[:, k * 128:(k + 1) * 128],
                                    xsp_b[:, k * 128:(k + 1) * 128], identb[:])
            psum_copy(xspT[:, :, :], ps[:].rearrange("p (k c) -> p k c", k=KC))

            # pre-GEMM: gates0[m*128+p, b*16+t] = (W_ih @ x_span.T)[...] + bias
            for m in range(GM):
                pg = psr.tile([128, 128], F32, tag="pr_if")
                for k in range(KC):
                    nc.tensor.matmul(pg[:],
                                     lhsT=WihT[:, k, m * 128:(m + 1) * 128],
                                     rhs=xspT[:, k, :],
                                     start=(k == 0), stop=(k == KC - 1))
                nc.vector.tensor_scalar_add(gates0[:, m * 128:(m + 1) * 128],
                                            pg[:], brow_c[:, m:m + 1])

            load_wT(WhhT, W_hh, GM, "wstg")

            # Wh/Wt -> whwt[p, k, 0:102] = [Wh.T | Wt.T] bf16
            for src, half in ((Wh, 0), (Wt, 1)):
                wp = wstage.tile([64, H], BF16, tag="wstg_w")
                nc.vector.memset(wp[:], 0.0)
                nc.gpsimd.dma_start(wp[0:R, :], src[:, :])
                ps = ptp.tile([128, H], BF16, tag="tp")
                for k in range(KC):
                    nc.tensor.transpose(ps[:, k * 64:(k + 1) * 64],
                                        wp[:, k * 128:(k + 1) * 128],
                                        identb[0:64, 0:64])
                psum_copy(
                    wt_tp[:, :, half * 64:half * 64 + 64],
                    ps[:, 0:KC * 64].rearrange("p (k c) -> p k c", k=KC))
            for k in range(KC):
                nc.vector.tensor_copy(
                    whwt[:, k, :].rearrange("p (w r) -> p w r", w=2),
                    wt_tp[:, k, :].rearrange("p (w r) -> p w r", w=2)[:, :, 0:R])

            # -------- recurrence + interleaved x-tile pipeline --------
            def x_tile(i):
                # DMA 2 tiles at once (bf16 cast in DMA)
                if i % 2 == 0:
                    b, s0 = i // 4, (i % 4) * 128
                    xb2 = xstage.tile([128, 2, H], BF16, tag="xb2")
                    nc.gpsimd.dma_start(
                        xb2[:],
                        embed[b, s0:s0 + 256, :]
                        .rearrange("(c p) h -> p c h", p=128))
                    x_tile.cur = xb2
                xb = x_tile.cur[:, i % 2, :]
                bns = xstage.tile([128, 12], F32, tag="bns")
                nc.vector.bn_stats(bns[:, 0:6], xb[:, 0:384])
                nc.vector.bn_stats(bns[:, 6:12], xb[:, 384:768])
                nc.vector.bn_aggr(
                    stats[:, :].rearrange("p (x i) -> p i x", x=2)[:, i, :],
                    bns[:])
                # xn = x - mu (per-partition scalar); /std rides the classifier
                xn = xstage.tile([128, H], BF16, tag="xn")
                nc.vector.tensor_scalar_sub(xn[:], xb, stats[:, i:i + 1])
                ps = ptp.tile([128, H], BF16, tag="tp")
                for k in range(KC):
                    nc.tensor.transpose(ps[:, k * 128:(k + 1) * 128],
                                        xn[:, k * 128:(k + 1) * 128], identb[:])
                psum_copy(xT_all[:, i, :, :],
                          ps[:].rearrange("p (k c) -> p k c", k=KC))

            for t in range(T):
                if t == 0:
                    # h0 = 0: gates come straight from the pre-GEMM
                    nc.scalar.activation(gsig[:, 0:2 * NKB],
                                         g0r[:, 0:12, :, 0], Sig)
                    nc.scalar.activation(gsig[:, 3 * NKB:4 * NKB],
                                         g0r[:, 12:18, :, 0], Tanh)
                    nc.scalar.activation(gsig[:, 2 * NKB:3 * NKB],
                                         g0r[:, 18:24, :, 0], Sig)
                else:
                    # i/f, g, o chunks in separate PSUM tiles so each add+
                    # activation starts as soon as its own chunks finish
                    pr_if = psr.tile([128, 2 * NKB], F32, tag="pr_if")
                    pr_g = psr.tile([128, NKB], F32, tag="pr_g")
                    pr_o = psr.tile([128, NKB], F32, tag="pr_o")
                    for m in range(12):
                        for k in range(KC):
                            nc.tensor.matmul(
                                pr_if[:, m * BL:(m + 1) * BL],
                                lhsT=WhhT[:, k, m * 128:(m + 1) * 128],
                                rhs=hbf[:, k * BL:(k + 1) * BL],
                                start=(k == 0), stop=(k == KC - 1))
                    nc.vector.tensor_add(
                        gates_f[:, 0:2 * NKB].rearrange("p (m b) -> p m b", m=12),
                        pr_if[:].rearrange("p (m b) -> p m b", m=12),
                        g0r[:, 0:12, :, t])
                    nc.scalar.activation(gsig[:, 0:2 * NKB],
                                         gates_f[:, 0:2 * NKB], Sig)
                    for m in range(12, 18):
                        for k in range(KC):
                            nc.tensor.matmul(
                                pr_g[:, (m - 12) * BL:(m - 11) * BL],
                                lhsT=WhhT[:, k, m * 128:(m + 1) * 128],
                                rhs=hbf[:, k * BL:(k + 1) * BL],
                                start=(k == 0), stop=(k == KC - 1))
                    nc.vector.tensor_add(
                        gates_f[:, 2 * NKB:3 * NKB]
                        .rearrange("p (m b) -> p m b", m=6),
                        pr_g[:].rearrange("p (m b) -> p m b", m=6),
                        g0r[:, 12:18, :, t])
                    nc.scalar.activation(gsig[:, 3 * NKB:4 * NKB],
                                         gates_f[:, 2 * NKB:3 * NKB], Tanh)
                    for m in range(18, GM):
                        for k in range(KC):
                            nc.tensor.matmul(
                                pr_o[:, (m - 18) * BL:(m - 17) * BL],
                                lhsT=WhhT[:, k, m * 128:(m + 1) * 128],
                                rhs=hbf[:, k * BL:(k + 1) * BL],
                                start=(k == 0), stop=(k == KC - 1))
                msk = mask_bc6[:, t * NKB:(t + 1) * NKB]
                # c_new = sig_f*c + sig_i*tanh_g ; c = where(mask, c_new, c)
                # (runs during the o-chunk matmuls; tanh(c_new) too)
                nc.vector.tensor_mul(tmp1[:], gsig[:, NKB:2 * NKB], cT[:])
                nc.vector.tensor_mul(tmp2[:], gsig[:, 0:NKB],
                                     gsig[:, 3 * NKB:4 * NKB])
                nc.vector.tensor_add(tmp1[:], tmp1[:], tmp2[:])
                nc.vector.copy_predicated(cT[:], msk, tmp1[:])
                nc.scalar.activation(tmp2[:], tmp1[:], Tanh)
                if t > 0:
                    nc.vector.tensor_add(
                        gates_f[:, 3 * NKB:4 * NKB]
                        .rearrange("p (m b) -> p m b", m=6),
                        pr_o[:].rearrange("p (m b) -> p m b", m=6),
                        g0r[:, 18:24, :, t])
                    nc.scalar.activation(gsig[:, 2 * NKB:3 * NKB],
                                         gates_f[:, 3 * NKB:4 * NKB], Sig)
                # h_new = sig_o * tanh(c_new) ; h = where(mask, h_new, h)
                # split in halves so the next step's first k-chunks start early
                HB = NKB // 2
                nc.vector.tensor_mul(tmph[:, 0:HB], gsig[:, 2 * NKB:2 * NKB + HB],
                                     tmp2[:, 0:HB])
                nc.vector.copy_predicated(hbf[:, 0:HB], msk[:, 0:HB],
                                          tmph[:, 0:HB])
                nc.vector.tensor_mul(tmph[:, HB:NKB],
                                     gsig[:, 2 * NKB + HB:3 * NKB],
                                     tmp2[:, HB:NKB])
                nc.vector.copy_predicated(hbf[:, HB:NKB], msk[:, HB:NKB],
                                          tmph[:, HB:NKB])

                # x-tiles trail by 3 steps (DMA data queued behind weights)
                # and sit at the END of the burst so they never delay the
                # gate chain
                if t >= 3:
                    x_tile(2 * (t - 3))
                    x_tile(2 * (t - 3) + 1)
                if t == 10:
                    # cln weights (reuse WihT's slot -- free after pre-GEMM);
                    # DMAs queue behind the x tiles already emitted, transposes
                    # fill PE gaps of the remaining steps
                    WwbT = pp.tile([128, KC, 2 * H], BF16, tag="WihT")
                    WwT = WwbT[:, :, 0:H]
                    WbT = WwbT[:, :, H:2 * H]
                    load_wT(WwT, cln_Ww, KC, "wstg")
                    load_wT(WbT, cln_Wb, KC, "wstg")

            for i in range(2 * (T - 3), NT):
                x_tile(i)

        with (
            tc.tile_pool(name="psum_small", bufs=2, space="PSUM") as pss,
            tc.tile_pool(name="psum_out", bufs=4, space="PSUM") as pso,
        ):
            # ---------------- CLN projections ----------------
            wT = pp.tile([128, NKB], F32, tag="wT")
            bT = pp.tile([128, NKB], F32, tag="bT")
            for dst, wmat, aff in ((wT, WwT, clnw_c), (bT, WbT, clnb_c)):
                for ko in range(KC):
                    ps = pss.tile([128, BL], F32, tag="ps_small")
                    for ki in range(KC):
                        nc.tensor.matmul(ps[:],
                                         lhsT=wmat[:, ki, ko * 128:(ko + 1) * 128],
                                         rhs=hbf[:, ki * BL:(ki + 1) * BL],
                                         start=(ki == 0), stop=(ki == KC - 1))
                    nc.vector.tensor_scalar_add(dst[:, ko * BL:(ko + 1) * BL],
                                                ps[:], aff[:, ko:ko + 1])
            bTb = pp.tile([128, NKB], BF16, tag="bTb")
            nc.vector.tensor_copy(bTb[:], bT[:])

            # ---------------- per-batch classifier params ----------------
            rhs_all = pp.tile([128, KC, BL, R2], BF16, tag="rhs_all")
            wTb = pp.tile([128, NKB], BF16, tag="wTb")
            nc.vector.tensor_copy(wTb[:], wT[:])
            nc.vector.tensor_tensor(
                rhs_all[:, :, :, :],
                whwt[:, :, :].rearrange("p k r -> p k () r")
                .to_broadcast([128, KC, BL, R2]),
                wTb[:, :].rearrange("p (k b) -> p k b ()", k=KC)
                .to_broadcast([128, KC, BL, R2]),
                op=mybir.AluOpType.mult)
            # c_b = [Wh|Wt] @ bvec_b + [bh|bt], broadcast across partitions
            c_all = pp.tile([1, BL * R2], F32, tag="c_all")
            c_bc = pp.tile([128, BL, R2], F32, tag="c_bc")
            for b in range(BL):
                ps2 = pss.tile([1, R2], F32, tag="ps_row")
                for k in range(KC):
                    nc.tensor.matmul(ps2[:],
                                     lhsT=bTb[:, k * BL + b:k * BL + b + 1],
                                     rhs=whwt[:, k, :], start=(k == 0), stop=False)
                nc.tensor.matmul(ps2[:], lhsT=ones_row[0:1, 0:1],
                                 rhs=bhbt[0:1, :], start=False, stop=True)
                nc.scalar.activation(c_all[0:1, b * R2:(b + 1) * R2], ps2[:], Copy)
            nc.gpsimd.partition_broadcast(
                c_bc[:, :, :].rearrange("p b r -> p (b r)"), c_all[0:1, :])

            # stats finalize: one batched sqrt + reciprocal (one table swap)
            nc.scalar.activation(stats[:, NT:2 * NT], stats[:, NT:2 * NT], Sqrt,
                                 bias=eps_t[:, 0:1])
            nc.vector.reciprocal(a_all[:], stats[:, NT:2 * NT])

            # ---------------- classifier ----------------
            # logits = a * (xn @ Wc.T) + c   (xn = x - mu)
            for i in range(NT):
                b, s0 = i // 4, (i % 4) * 128
                pt = pso.tile([128, R2], F32, tag="pt")
                for k in range(KC):
                    nc.tensor.matmul(pt[:], lhsT=xT_all[:, i, k, :],
                                     rhs=rhs_all[:, k, b, :],
                                     start=(k == 0), stop=(k == KC - 1))
                q = xstage.tile([128, R2], F32, tag="q")
                nc.vector.tensor_scalar_mul(q[:], pt[:], a_all[:, i:i + 1])
                nc.vector.tensor_add(q[:], q[:], c_bc[:, b, :])
                if i % 2 == 0:
                    out2 = ostage.tile([128, 2, R2], F32, tag="out2")
                nc.scalar.activation(out2[:, i % 2, :], q[:], Sig)
                if i % 2 == 1:
                    c2 = (i % 4) // 2
                    hv = heads[b, :, :].rearrange("(c p) r -> p c r", p=128)
                    tv = tails[b, :, :].rearrange("(c p) r -> p c r", p=128)
                    nc.sync.dma_start(hv[:, 2 * c2:2 * c2 + 2, :],
                                      out2[:, :, 0:R])
                    nc.sync.dma_start(tv[:, 2 * c2:2 * c2 + 2, :],
                                      out2[:, :, R:R2])


@functools.cache
def _build():
    nc = bacc.Bacc("TRN2", target_bir_lowering=False, debug=False,
                   enable_asserts=False, num_devices=NCORES)
    with tile.TileContext(nc) as tc:
        _kernel_body(tc)
    nc.compile()
    return nc


def kernel(**inputs):
    nc = _build()
    shared = {k: np.ascontiguousarray(np.asarray(inputs[k], dtype=np.float32))
              for k in ("W_ih", "W_hh", "b_ih", "b_hh", "cln_weight", "cln_bias",
                        "cln_Ww", "cln_Wb", "Wh", "bh", "Wt", "bt")}
    embed = np.ascontiguousarray(np.asarray(inputs["embed"], dtype=np.float32))
    sh = np.ascontiguousarray(np.asarray(inputs["sub_head"], dtype=np.int32))
    st = np.ascontiguousarray(np.asarray(inputs["sub_tail"], dtype=np.int32))
    in_maps = []
    for c in range(NCORES):
        sl = slice(c * BL, (c + 1) * BL)
        in_maps.append(dict(shared, embed=np.ascontiguousarray(embed[sl]),
                            sub_head=np.ascontiguousarray(sh[sl]),
                            sub_tail=np.ascontiguousarray(st[sl])))
    res = run_bass_kernel_spmd(nc, in_maps, list(range(NCORES)),
                               trace=bool(int(os.environ.get("KTRACE", "0"))))
    heads = np.concatenate([r["heads"] for r in res.results], axis=0)
    tails = np.concatenate([r["tails"] for r in res.results], axis=0)
    kernel.last_exec_time_ns = res.exec_time_ns
    return heads, tails


if __name__ == "__main__":
    np.random.seed(0)
    ins = {
        "embed": np.random.randn(B, S, H).astype(np.float32),
        "sub_head": np.random.randint(0, S - T, size=(B,)).astype(np.int32),
        "W_ih": (np.random.randn(4 * H, H) * 0.02).astype(np.float32),
        "W_hh": (np.random.randn(4 * H, H) * 0.02).astype(np.float32),
        "b_ih": np.zeros(4 * H, np.float32),
        "b_hh": np.zeros(4 * H, np.float32),
        "cln_weight": np.ones(H, np.float32),
        "cln_bias": np.zeros(H, np.float32),
        "cln_Ww": (np.random.randn(H, H) * 0.02).astype(np.float32),
        "cln_Wb": (np.random.randn(H, H) * 0.02).astype(np.float32),
        "Wh": (np.random.randn(R, H) * 0.02).astype(np.float32),
        "bh": np.zeros(R, np.float32),
        "Wt": (np.random.randn(R, H) * 0.02).astype(np.float32),
        "bt": np.zeros(R, np.float32),
    }
    ins["sub_tail"] = (ins["sub_head"]
                       + np.random.randint(0, T, size=(B,))).astype(np.int32)
    h, t = kernel(**ins)
    print("ok", h.shape, t.shape, h.dtype)


# revision 32
# speedup vs baseline: 1.0376x; 1.0376x over previous
"""CasRel-style kernel for Trainium2 (Bass/Tile), 8-core data-parallel.

Model (per batch b):
  hn_b      = masked LSTM over embed[b, head_b : head_b+16, :]  (16 steps, H=768)
  w_b       = hn_b @ cln_Ww.T + cln_weight ; bvec_b = hn_b @ cln_Wb.T + cln_bias
  ne[b,s]   = w_b * (x - mu)/std + bvec_b          (layernorm over H)
  heads     = sigmoid(ne @ Wh.T + bh) ; tails = sigmoid(ne @ Wt.T + bt)

Kernel strategy (per core, 8 local batches):
  - LN + classifier folded into one PSUM accumulation on raw x:
      logits = a_s * (x_s @ Wc_b.T - mu_s x vh_b + std_s x c_b),  a_s = 1/std_s
    where Wc_b = [Wh|Wt] col-scaled by w_b, vh_b = Wc_b @ 1,
    c_b = [Wh|Wt] @ bvec_b + [bh|bt]. The mu/std rank-1 corrections ride into
    PSUM as K=1 matmuls (mu/std as bf16 rows); a_s rides the sigmoid scale.
  - All matmul operands bf16; weights cast to bf16 during DMA (SWDGE) and
    transposed on-chip by the PE (bf16 transpose -> bf16 PSUM -> one batched
    copy per 768 columns). No xbar DMA transposes anywhere.
  - All HBM reads go through the gpsimd SWDGE queue in priority order
    (gather, W_ih, W_hh, Wh/Wt, x tiles, cln_Ww/Wb) so the recurrence's
    weights are never starved by embed traffic.
  - LSTM: x-part pre-GEMMed for all 16 steps; per-step h @ W_hh.T uses two
    PSUM tiles (i/f/g and o) so the i/f/g activations overlap the o matmuls.
  - x-tile pipeline (stats + PE transposes) interleaved 2 tiles per LSTM step.
"""

import functools
import os
import sys

import numpy as np

sys.path.insert(0, "/opt/trn_rl_repo")

import concourse.bass as bass
import concourse.tile as tile
from concourse import bacc, mybir
from concourse.bass import IndirectOffsetOnAxis
from concourse.bass_utils import run_bass_kernel_spmd
from concourse.masks import make_identity

F32 = mybir.dt.float32
BF16 = mybir.dt.bfloat16
I32 = mybir.dt.int32

B, S, H, R = 64, 512, 768, 51
T = 16                      # MAX_SPAN
NCORES = 8
BL = B // NCORES            # 8 local batches
KC = H // 128               # 6 contraction chunks
GM = 4 * H // 128           # 24 gate-dim chunks
NT = BL * S // 128          # 32 token tiles per core
R2 = 2 * R                  # 102 combined head+tail outputs
EPS = 1e-12


def _kernel_body(tc):
    nc = tc.nc
    embed = nc.dram_tensor("embed", [BL, S, H], F32, kind="ExternalInput").ap()
    sub_head = nc.dram_tensor("sub_head", [BL], I32, kind="ExternalInput").ap()
    sub_tail = nc.dram_tensor("sub_tail", [BL], I32, kind="ExternalInput").ap()
    W_ih = nc.dram_tensor("W_ih", [4 * H, H], F32, kind="ExternalInput").ap()
    W_hh = nc.dram_tensor("W_hh", [4 * H, H], F32, kind="ExternalInput").ap()
    b_ih = nc.dram_tensor("b_ih", [4 * H], F32, kind="ExternalInput").ap()
    b_hh = nc.dram_tensor("b_hh", [4 * H], F32, kind="ExternalInput").ap()
    cln_w = nc.dram_tensor("cln_weight", [H], F32, kind="ExternalInput").ap()
    cln_b = nc.dram_tensor("cln_bias", [H], F32, kind="ExternalInput").ap()
    cln_Ww = nc.dram_tensor("cln_Ww", [H, H], F32, kind="ExternalInput").ap()
    cln_Wb = nc.dram_tensor("cln_Wb", [H, H], F32, kind="ExternalInput").ap()
    Wh = nc.dram_tensor("Wh", [R, H], F32, kind="ExternalInput").ap()
    bh = nc.dram_tensor("bh", [R], F32, kind="ExternalInput").ap()
    Wt = nc.dram_tensor("Wt", [R, H], F32, kind="ExternalInput").ap()
    bt = nc.dram_tensor("bt", [R], F32, kind="ExternalInput").ap()
    heads = nc.dram_tensor("heads", [BL, S, R], F32, kind="ExternalOutput").ap()
    tails = nc.dram_tensor("tails", [BL, S, R], F32, kind="ExternalOutput").ap()

    Sig = mybir.ActivationFunctionType.Sigmoid
    Tanh = mybir.ActivationFunctionType.Tanh
    Sqrt = mybir.ActivationFunctionType.Sqrt
    Copy = mybir.ActivationFunctionType.Copy
    NKB = KC * BL  # 48

    # alternate PSUM->SBUF copies between the two streaming engines
    _ccnt = [0]

    def psum_copy(dst, src):
        if _ccnt[0] % 2 == 0:
            nc.scalar.activation(dst, src, Copy)
        else:
            nc.vector.tensor_copy(dst, src)
        _ccnt[0] += 1

    with (
        tc.tile_pool(name="persist", bufs=1) as pp,
        tc.tile_pool(name="wstage", bufs=3) as wstage,
        tc.tile_pool(name="xstage", bufs=2) as xstage,
        tc.tile_pool(name="ostage", bufs=3) as ostage,
        tc.tile_pool(name="xnpool", bufs=6) as xnpool,
    ):
        # ---------------- constants / small prep ----------------
        identb = pp.tile([128, 128], BF16, tag="identb")
        make_identity(nc, identb[:])

        ones_row = pp.tile([1, 128], BF16, tag="ones_row")
        nc.vector.memset(ones_row[:], 1.0)

        head_sb = pp.tile([BL, 1], I32, tag="head_sb")
        tail_sb = pp.tile([BL, 1], I32, tag="tail_sb")
        nc.sync.dma_start(head_sb[:], sub_head.rearrange("b -> b ()"))
        nc.sync.dma_start(tail_sb[:], sub_tail.rearrange("b -> b ()"))

        # gather row offsets: off[b, t] = b*S + head_b + t  (embed flat [BL*S, H])
        iota_bs = pp.tile([BL, T], I32, tag="iota_bs")
        nc.gpsimd.iota(iota_bs[:], pattern=[[1, T]], base=0, channel_multiplier=S)
        off_bt = pp.tile([BL, T], I32, tag="off_bt")
        nc.vector.tensor_tensor(off_bt[:], iota_bs[:],
                                head_sb[:, 0:1].to_broadcast([BL, T]),
                                op=mybir.AluOpType.add)
        off_p = pp.tile([128, 1], I32, tag="off_p")
        nc.sync.dma_start(off_p[:], off_bt[:])

        # span mask: mask[b, t] = (t <= tail_b - head_b)
        span = pp.tile([BL, 1], I32, tag="span")
        nc.vector.tensor_tensor(span[:], tail_sb[:], head_sb[:],
                                op=mybir.AluOpType.subtract)
        iota_t = pp.tile([BL, T], I32, tag="iota_t")
        nc.gpsimd.iota(iota_t[:], pattern=[[1, T]], base=0, channel_multiplier=0)
        mask_pad = pp.tile([32, 32], I32, tag="mask_pad")
        nc.vector.memset(mask_pad[:], 0)
        nc.vector.tensor_tensor(mask_pad[0:BL, 0:T], iota_t[:],
                                span[:, 0:1].to_broadcast([BL, T]),
                                op=mybir.AluOpType.is_le)
        maskTt = pp.tile([32, 32], I32, tag="maskTt")      # [t, b]
        nc.vector.transpose(maskTt[:], mask_pad[:])
        maskT = pp.tile([1, T * BL], I32, tag="maskT")     # col = t*8+b
        nc.sync.dma_start(maskT[0:1, :], maskTt[0:T, 0:BL])
        mask_bc = pp.tile([128, T * BL], I32, tag="mask_bc")
        nc.gpsimd.partition_broadcast(mask_bc[:], maskT[0:1, :])
        # replicate over h-chunks: mask_bc6[p, t*48 + k*8 + b] = mask[b, t]
        mask_bc6 = pp.tile([128, T * KC * BL], mybir.dt.uint8, tag="mask_bc6")
        m6v = mask_bc6[:, :].rearrange("p (t k b) -> p k t b", k=KC, b=BL)
        mbv = mask_bc[:, :].rearrange("p (t b) -> p t b", b=BL)
        for k in range(KC):
            nc.vector.tensor_copy(m6v[:, k], mbv)

        # chunk-major small vectors: v_c[p, m] = v[m*128 + p]
        clnw_c = pp.tile([128, KC], F32, tag="clnw_c")
        clnb_c = pp.tile([128, KC], F32, tag="clnb_c")
        nc.sync.dma_start(clnw_c[:], cln_w.rearrange("(k p) -> p k", p=128))
        nc.sync.dma_start(clnb_c[:], cln_b.rearrange("(k p) -> p k", p=128))
        brow_c = pp.tile([128, GM], F32, tag="brow_c")
        nc.gpsimd.dma_start(brow_c[:], b_ih.rearrange("(m p) -> p m", p=128))
        nc.gpsimd.dma_start(brow_c[:], b_hh.rearrange("(m p) -> p m", p=128),
                            accum_op=mybir.AluOpType.add)
        bhbt_f = pp.tile([1, R2], F32, tag="bhbt_f")
        nc.sync.dma_start(bhbt_f[:, 0:R], bh.rearrange("r -> () r"))
        nc.sync.dma_start(bhbt_f[:, R:R2], bt.rearrange("r -> () r"))
        bhbt = pp.tile([1, R2], BF16, tag="bhbt")
        nc.vector.tensor_copy(bhbt[:], bhbt_f[:])

        # span gather (f32) -> bf16 for the pre-GEMM rhs
        xsp_f = xstage.tile([128, H], F32, tag="xsp_f")
        nc.gpsimd.indirect_dma_start(
            out=xsp_f[:], out_offset=None,
            in_=embed.rearrange("b s h -> (b s) h"),
            in_offset=IndirectOffsetOnAxis(ap=off_p[:, 0:1], axis=0))
        xsp_b = xstage.tile([128, H], BF16, tag="xsp_b")
        nc.vector.tensor_copy(xsp_b[:], xsp_f[:])

        # persistent big tensors
        WihT = pp.tile([128, KC, 4 * H], BF16, tag="WihT")
        WhhT = pp.tile([128, KC, 4 * H], BF16, tag="WhhT")
        xT_all = pp.tile([128, NT, KC, 128], BF16, tag="xT_all")
        gates0 = pp.tile([128, GM * 128], BF16, tag="gates0")
        g0r = gates0[:, :].rearrange("p (m b t) -> p m b t", m=GM, b=BL)
        xspT = pp.tile([128, KC, 128], BF16, tag="xspT")
        whwt = pp.tile([128, KC, R2], BF16, tag="whwt")
        wt_tp = pp.tile([128, KC, 128], BF16, tag="wt_tp")

        # LSTM state
        hbf = pp.tile([128, NKB], BF16, tag="hbf")   # col = k*8 + b
        cT = pp.tile([128, NKB], F32, tag="cT")
        gsig = pp.tile([128, 4 * NKB], F32, tag="gsig")  # i|f|o|tanh_g
        gates_f = pp.tile([128, 4 * NKB], F32, tag="gates_f")
        tmp1 = pp.tile([128, NKB], F32, tag="tmp1")
        tmp2 = pp.tile([128, NKB], F32, tag="tmp2")
        tmph = pp.tile([128, NKB], BF16, tag="tmph")
        nc.vector.memset(hbf[:], 0.0)
        nc.vector.memset(cT[:], 0.0)

        # stats: col i = mu_i, col NT+i = var_i (later std_i)
        stats = pp.tile([128, 2 * NT], F32, tag="stats")
        a_all = pp.tile([128, NT], F32, tag="a_all")
        eps_t = pp.tile([128, 1], F32, tag="eps_t")
        nc.vector.memset(eps_t[:], EPS)

        with (
            tc.tile_pool(name="psum_tp", bufs=2, space="PSUM") as ptp,
            tc.tile_pool(name="psum_rec", bufs=2, space="PSUM") as psr,
        ):
            # -------- weight load (bf16 cast in DMA) + PE transpose --------
            def load_wT(dst, src_ap, nm, stag):
                # dst[:, k, m*128 + c] = src[m*128 + c, k*128 + p]
                for m0 in range(0, nm, 2):
                    st = wstage.tile([128, 2, H], BF16, tag=stag)
                    nc.gpsimd.dma_start(
                        st[:, :, :],
                        src_ap[m0 * 128:(m0 + 2) * 128, :]
                        .rearrange("(c p) h -> p c h", p=128))
                    for c in range(2):
                        ps = ptp.tile([128, H], BF16, tag="tp")
                        for k in range(KC):
                            nc.tensor.transpose(ps[:, k * 128:(k + 1) * 128],
                                                st[:, c, k * 128:(k + 1) * 128],
                                                identb[:])
                        m = m0 + c
                        psum_copy(
                            dst[:, :, m * 128:(m + 1) * 128],
                            ps[:].rearrange("p (k c) -> p k c", k=KC))

            # W_ih first (pre-GEMM gates t=0), then W_hh (recurrence t>=1)
            load_wT(WihT, W_ih, GM, "wstg")

            # xspT: 6 transposes of the gathered span block
            ps = ptp.tile([128, H], BF16, tag="tp")
            for k in range(KC):
                nc.tensor.transpose(ps[:, k * 128:(k + 1) * 128],
                                    xsp_b[:, k * 128:(k + 1) * 128], identb[:])
            psum_copy(xspT[:, :, :], ps[:].rearrange("p (k c) -> p k c", k=KC))

            # pre-GEMM: gates0[m*128+p, b*16+t] = (W_ih @ x_span.T)[...] + bias
            for m in range(GM):
                pg = psr.tile([128, 128], F32, tag="pr_if")
                for k in range(KC):
                    nc.tensor.matmul(pg[:],
                                     lhsT=WihT[:, k, m * 128:(m + 1) * 128],
                                     rhs=xspT[:, k, :],
                                     start=(k == 0), stop=(k == KC - 1))
                nc.vector.tensor_scalar_add(gates0[:, m * 128:(m + 1) * 128],
                                            pg[:], brow_c[:, m:m + 1])

            load_wT(WhhT, W_hh, GM, "wstg")

            # Wh/Wt -> whwt[p, k, 0:102] = [Wh.T | Wt.T] bf16
            for src, half in ((Wh, 0), (Wt, 1)):
                wp = wstage.tile([64, H], BF16, tag="wstg_w")
                nc.vector.memset(wp[:], 0.0)
                nc.gpsimd.dma_start(wp[0:R, :], src[:, :])
                ps = ptp.tile([128, H], BF16, tag="tp")
                for k in range(KC):
                    nc.tensor.transpose(ps[:, k * 64:(k + 1) * 64],
                                        wp[:, k * 128:(k + 1) * 128],
                                        identb[0:64, 0:64])
                psum_copy(
                    wt_tp[:, :, half * 64:half * 64 + 64],
                    ps[:, 0:KC * 64].rearrange("p (k c) -> p k c", k=KC))
            for k in range(KC):
                nc.vector.tensor_copy(
                    whwt[:, k, :].rearrange("p (w r) -> p w r", w=2),
                    wt_tp[:, k, :].rearrange("p (w r) -> p w r", w=2)[:, :, 0:R])

            # -------- recurrence + interleaved x-tile pipeline --------
            # split: stats phase (DMA + DVE) runs 2 steps ahead of the
            # transpose phase (PE) so the PE stream never waits on DVE
            xn_ring = {}

            def x_stats(i):
                # DMA 2 tiles at once (bf16 cast in DMA)
                if i % 2 == 0:
                    b, s0 = i // 4, (i % 4) * 128
                    xb2 = xstage.tile([128, 2, H], BF16, tag="xb2")
                    nc.gpsimd.dma_start(
                        xb2[:],
                        embed[b, s0:s0 + 256, :]
                        .rearrange("(c p) h -> p c h", p=128))
                    x_stats.cur = xb2
                xb = x_stats.cur[:, i % 2, :]
                bns = xstage.tile([128, 12], F32, tag="bns")
                nc.vector.bn_stats(bns[:, 0:6], xb[:, 0:384])
                nc.vector.bn_stats(bns[:, 6:12], xb[:, 384:768])
                nc.vector.bn_aggr(
                    stats[:, :].rearrange("p (x i) -> p i x", x=2)[:, i, :],
                    bns[:])
                # xn = x - mu (per-partition scalar); /std rides the classifier
                xn = xnpool.tile([128, H], BF16, tag="xn")
                nc.vector.tensor_scalar_sub(xn[:], xb, stats[:, i:i + 1])
                xn_ring[i] = xn

            def x_tp(i):
                xn = xn_ring.pop(i)
                ps = ptp.tile([128, H], BF16, tag="tp")
                for k in range(KC):
                    nc.tensor.transpose(ps[:, k * 128:(k + 1) * 128],
                                        xn[:, k * 128:(k + 1) * 128], identb[:])
                psum_copy(xT_all[:, i, :, :],
                          ps[:].rearrange("p (k c) -> p k c", k=KC))

            for t in range(T):
                if t == 0:
                    # h0 = 0: gates come straight from the pre-GEMM
                    nc.scalar.activation(gsig[:, 0:2 * NKB],
                                         g0r[:, 0:12, :, 0], Sig)
                    nc.scalar.activation(gsig[:, 3 * NKB:4 * NKB],
                                         g0r[:, 12:18, :, 0], Tanh)
                    nc.scalar.activation(gsig[:, 2 * NKB:3 * NKB],
                                         g0r[:, 18:24, :, 0], Sig)
                else:
                    # i/f, g, o chunks in separate PSUM tiles so each add+
                    # activation starts as soon as its own chunks finish
                    pr_if = psr.tile([128, 2 * NKB], F32, tag="pr_if")
                    pr_g = psr.tile([128, NKB], F32, tag="pr_g")
                    pr_o = psr.tile([128, NKB], F32, tag="pr_o")
                    for m in range(12):
                        for k in range(KC):
                            nc.tensor.matmul(
                                pr_if[:, m * BL:(m + 1) * BL],
                                lhsT=WhhT[:, k, m * 128:(m + 1) * 128],
                                rhs=hbf[:, k * BL:(k + 1) * BL],
                                start=(k == 0), stop=(k == KC - 1))
                    nc.vector.tensor_add(
                        gates_f[:, 0:2 * NKB].rearrange("p (m b) -> p m b", m=12),
                        pr_if[:].rearrange("p (m b) -> p m b", m=12),
                        g0r[:, 0:12, :, t])
                    nc.scalar.activation(gsig[:, 0:2 * NKB],
                                         gates_f[:, 0:2 * NKB], Sig)
                    for m in range(12, 18):
                        for k in range(KC):
                            nc.tensor.matmul(
                                pr_g[:, (m - 12) * BL:(m - 11) * BL],
                                lhsT=WhhT[:, k, m * 128:(m + 1) * 128],
                                rhs=hbf[:, k * BL:(k + 1) * BL],
                                start=(k == 0), stop=(k == KC - 1))
                    nc.vector.tensor_add(
                        gates_f[:, 2 * NKB:3 * NKB]
                        .rearrange("p (m b) -> p m b", m=6),
                        pr_g[:].rearrange("p (m b) -> p m b", m=6),
                        g0r[:, 12:18, :, t])
                    nc.scalar.activation(gsig[:, 3 * NKB:4 * NKB],
                                         gates_f[:, 2 * NKB:3 * NKB], Tanh)
                    for m in range(18, GM):
                        for k in range(KC):
                            nc.tensor.matmul(
                                pr_o[:, (m - 18) * BL:(m - 17) * BL],
                                lhsT=WhhT[:, k, m * 128:(m + 1) * 128],
                                rhs=hbf[:, k * BL:(k + 1) * BL],
                                start=(k == 0), stop=(k == KC - 1))
                msk = mask_bc6[:, t * NKB:(t + 1) * NKB]
                # c_new = sig_f*c + sig_i*tanh_g ; c = where(mask, c_new, c)
                # (runs during the o-chunk matmuls; tanh(c_new) too)
                nc.vector.tensor_mul(tmp1[:], gsig[:, NKB:2 * NKB], cT[:])
                nc.vector.tensor_mul(tmp2[:], gsig[:, 0:NKB],
                                     gsig[:, 3 * NKB:4 * NKB])
                nc.vector.tensor_add(tmp1[:], tmp1[:], tmp2[:])
                nc.vector.copy_predicated(cT[:], msk, tmp1[:])
                nc.scalar.activation(tmp2[:], tmp1[:], Tanh)
                if t > 0:
                    nc.vector.tensor_add(
                        gates_f[:, 3 * NKB:4 * NKB]
                        .rearrange("p (m b) -> p m b", m=6),
                        pr_o[:].rearrange("p (m b) -> p m b", m=6),
                        g0r[:, 18:24, :, t])
                    nc.scalar.activation(gsig[:, 2 * NKB:3 * NKB],
                                         gates_f[:, 3 * NKB:4 * NKB], Sig)
                # h_new = sig_o * tanh(c_new) ; h = where(mask, h_new, h)
                # split in halves so the next step's first k-chunks start early
                HB = NKB // 2
                nc.vector.tensor_mul(tmph[:, 0:HB], gsig[:, 2 * NKB:2 * NKB + HB],
                                     tmp2[:, 0:HB])
                nc.vector.copy_predicated(hbf[:, 0:HB], msk[:, 0:HB],
                                          tmph[:, 0:HB])
                nc.vector.tensor_mul(tmph[:, HB:NKB],
                                     gsig[:, 2 * NKB + HB:3 * NKB],
                                     tmp2[:, HB:NKB])
                nc.vector.copy_predicated(hbf[:, HB:NKB], msk[:, HB:NKB],
                                          tmph[:, HB:NKB])

                # x stats trail by 3 steps (DMA data queued behind weights);
                # transposes trail 2 further so their xn input is ready
                if t >= 3:
                    x_stats(2 * (t - 3))
                    x_stats(2 * (t - 3) + 1)
                if t >= 5:
                    x_tp(2 * (t - 5))
                    x_tp(2 * (t - 5) + 1)
                if t == 10:
                    # cln weights (reuse WihT's slot -- free after pre-GEMM);
                    # DMAs queue behind the x tiles already emitted, transposes
                    # fill PE gaps of the remaining steps
                    WwbT = pp.tile([128, KC, 2 * H], BF16, tag="WihT")
                    WwT = WwbT[:, :, 0:H]
                    WbT = WwbT[:, :, H:2 * H]
                    load_wT(WwT, cln_Ww, KC, "wstg")
                    load_wT(WbT, cln_Wb, KC, "wstg")

            for i in range(2 * (T - 3), NT):
                x_stats(i)
                x_tp(i - 4)
            for i in range(NT - 4, NT):
                x_tp(i)

        with (
            tc.tile_pool(name="psum_small", bufs=2, space="PSUM") as pss,
            tc.tile_pool(name="psum_out", bufs=4, space="PSUM") as pso,
        ):
            # ---------------- CLN projections ----------------
            wT = pp.tile([128, NKB], F32, tag="wT")
            bT = pp.tile([128, NKB], F32, tag="bT")
            for dst, wmat, aff in ((wT, WwT, clnw_c), (bT, WbT, clnb_c)):
                for ko in range(KC):
                    ps = pss.tile([128, BL], F32, tag="ps_small")
                    for ki in range(KC):
                        nc.tensor.matmul(ps[:],
                                         lhsT=wmat[:, ki, ko * 128:(ko + 1) * 128],
                                         rhs=hbf[:, ki * BL:(ki + 1) * BL],
                                         start=(ki == 0), stop=(ki == KC - 1))
                    nc.vector.tensor_scalar_add(dst[:, ko * BL:(ko + 1) * BL],
                                                ps[:], aff[:, ko:ko + 1])
            bTb = pp.tile([128, NKB], BF16, tag="bTb")
            nc.vector.tensor_copy(bTb[:], bT[:])

            # ---------------- per-batch classifier params ----------------
            rhs_all = pp.tile([128, KC, BL, R2], BF16, tag="rhs_all")
            wTb = pp.tile([128, NKB], BF16, tag="wTb")
            nc.vector.tensor_copy(wTb[:], wT[:])
            nc.vector.tensor_tensor(
                rhs_all[:, :, :, :],
                whwt[:, :, :].rearrange("p k r -> p k () r")
                .to_broadcast([128, KC, BL, R2]),
                wTb[:, :].rearrange("p (k b) -> p k b ()", k=KC)
                .to_broadcast([128, KC, BL, R2]),
                op=mybir.AluOpType.mult)
            # c_b = [Wh|Wt] @ bvec_b + [bh|bt], broadcast across partitions
            c_all = pp.tile([1, BL * R2], F32, tag="c_all")
            c_bc = pp.tile([128, BL, R2], F32, tag="c_bc")
            for b in range(BL):
                ps2 = pss.tile([1, R2], F32, tag="ps_row")
                for k in range(KC):
                    nc.tensor.matmul(ps2[:],
                                     lhsT=bTb[:, k * BL + b:k * BL + b + 1],
                                     rhs=whwt[:, k, :], start=(k == 0), stop=False)
                nc.tensor.matmul(ps2[:], lhsT=ones_row[0:1, 0:1],
                                 rhs=bhbt[0:1, :], start=False, stop=True)
                nc.scalar.activation(c_all[0:1, b * R2:(b + 1) * R2], ps2[:], Copy)
            nc.gpsimd.partition_broadcast(
                c_bc[:, :, :].rearrange("p b r -> p (b r)"), c_all[0:1, :])

            # stats finalize: one batched sqrt + reciprocal (one table swap)
            nc.scalar.activation(stats[:, NT:2 * NT], stats[:, NT:2 * NT], Sqrt,
                                 bias=eps_t[:, 0:1])
            nc.vector.reciprocal(a_all[:], stats[:, NT:2 * NT])

            # ---------------- classifier ----------------
            # logits = a * (xn @ Wc.T) + c   (xn = x - mu)
            for i in range(NT):
                b, s0 = i // 4, (i % 4) * 128
                pt = pso.tile([128, R2], F32, tag="pt")
                for k in range(KC):
                    nc.tensor.matmul(pt[:], lhsT=xT_all[:, i, k, :],
                                     rhs=rhs_all[:, k, b, :],
                                     start=(k == 0), stop=(k == KC - 1))
                q = xstage.tile([128, R2], F32, tag="q")
                nc.vector.tensor_scalar_mul(q[:], pt[:], a_all[:, i:i + 1])
                nc.vector.tensor_add(q[:], q[:], c_bc[:, b, :])
                if i % 2 == 0:
                    out2 = ostage.tile([128, 2, R2], F32, tag="out2")
                nc.scalar.activation(out2[:, i % 2, :], q[:], Sig)
                if i % 2 == 1:
                    c2 = (i % 4) // 2
                    hv = heads[b, :, :].rearrange("(c p) r -> p c r", p=128)
                    tv = tails[b, :, :].rearrange("(c p) r -> p c r", p=128)
                    nc.sync.dma_start(hv[:, 2 * c2:2 * c2 + 2, :],
                                      out2[:, :, 0:R])
                    nc.sync.dma_start(tv[:, 2 * c2:2 * c2 + 2, :],
                                      out2[:, :, R:R2])


@functools.cache
def _build():
    nc = bacc.Bacc("TRN2", target_bir_lowering=False, debug=False,
                   enable_asserts=False, num_devices=NCORES)
    with tile.TileContext(nc) as tc:
        _kernel_body(tc)
    nc.compile()
    return nc


def kernel(**inputs):
    nc = _build()
    shared = {k: np.ascontiguousarray(np.asarray(inputs[k], dtype=np.float32))
              for k in ("W_ih", "W_hh", "b_ih", "b_hh", "cln_weight", "cln_bias",
                        "cln_Ww", "cln_Wb", "Wh", "bh", "Wt", "bt")}
    embed = np.ascontiguousarray(np.asarray(inputs["embed"], dtype=np.float32))
    sh = np.ascontiguousarray(np.asarray(inputs["sub_head"], dtype=np.int32))
    st = np.ascontiguousarray(np.asarray(inputs["sub_tail"], dtype=np.int32))
    in_maps = []
    for c in range(NCORES):
        sl = slice(c * BL, (c + 1) * BL)
        in_maps.append(dict(shared, embed=np.ascontiguousarray(embed[sl]),
                            sub_head=np.ascontiguousarray(sh[sl]),
                            sub_tail=np.ascontiguousarray(st[sl])))
    res = run_bass_kernel_spmd(nc, in_maps, list(range(NCORES)),
                               trace=bool(int(os.environ.get("KTRACE", "0"))))
    heads = np.concatenate([r["heads"] for r in res.results], axis=0)
    tails = np.concatenate([r["tails"] for r in res.results], axis=0)
    kernel.last_exec_time_ns = res.exec_time_ns
    return heads, tails


if __name__ == "__main__":
    np.random.seed(0)
    ins = {
        "embed": np.random.randn(B, S, H).astype(np.float32),
        "sub_head": np.random.randint(0, S - T, size=(B,)).astype(np.int32),
        "W_ih": (np.random.randn(4 * H, H) * 0.02).astype(np.float32),
        "W_hh": (np.random.randn(4 * H, H) * 0.02).astype(np.float32),
        "b_ih": np.zeros(4 * H, np.float32),
        "b_hh": np.zeros(4 * H, np.float32),
        "cln_weight": np.ones(H, np.float32),
        "cln_bias": np.zeros(H, np.float32),
        "cln_Ww": (np.random.randn(H, H) * 0.02).astype(np.float32),
        "cln_Wb": (np.random.randn(H, H) * 0.02).astype(np.float32),
        "Wh": (np.random.randn(R, H) * 0.02).astype(np.float32),
        "bh": np.zeros(R, np.float32),
        "Wt": (np.random.randn(R, H) * 0.02).astype(np.float32),
        "bt": np.zeros(R, np.float32),
    }
    ins["sub_tail"] = (ins["sub_head"]
                       + np.random.randint(0, T, size=(B,))).astype(np.int32)
    h, t = kernel(**ins)
    print("ok", h.shape, t.shape, h.dtype)
